# revision 9
# baseline (speedup 1.0000x reference)
"""KoLeo-loss kernel v6 for 8 Trainium2 NeuronCores.

v3 -> v6:
  - Explicit double-buffered PSUM slots (no pool rotation) so the LSE
    stream ping-pongs psA/psB with zero slot-parity breaks.
  - Block-4 row-max work is split into 512-col pieces (plus a few
    1024-col doubles) that ride in the *opposite* slot's tail region
    [2048-w:2048) right after that slot's activation finished.  The
    piece's DVE reduce only gates the last 512-col span of the next
    gram into that slot, so ACT never stalls on block-4 work.
  - ident/warmup emitted first; input DMA chunk order prioritizes
    blk0's upper half and block-4 columns.
  - Final column-sum DMA split across the sync and scalar hardware
    queues to shorten the drain tail.

Scheme (unchanged from v3): circulant cover, per-core 2048 rows.
Block 0 upper-triangular LSE with -240 diag kill; blocks 1..3 full
LSE (exp row-accum to srow, DVE column accumulation to scol); block 4
covered by both endpoint cores, row-max only (rmax, 4 slots per qi,
host max-combines).  Host: S = srow + scol contributions, est =
(log S + BIAS)/BETA, gram-max = max(est, rmax/256), koleo loss.
"""

import sys

if "/opt/trn_rl_repo" not in sys.path:
    sys.path.insert(0, "/opt/trn_rl_repo")

import numpy as np

P = 128
D = 256
B_FULL = 16384
N_CORES = 8
BLK = 2048
NLSE = 4  # blocks 0..3 via LSE
NQ = 16

BETA = 768.0
BIAS = 227.0
SCALE_IN = 16.0
ACT_SCALE = BETA / (SCALE_IN * SCALE_IN)
RMAX_SLOTS = 4  # per-qi block-4 partial-max slots (512 cols each)


def build_nc():
    import concourse.mybir as mybir
    import concourse.tile as tile
    from concourse import bacc
    from concourse.masks import make_identity

    dt = mybir.dt
    AF = mybir.ActivationFunctionType
    OP = mybir.AluOpType
    DR = mybir.MatmulPerfMode.DoubleRow

    nc = bacc.Bacc(None)
    xq_in = nc.declare_dram_parameter("xq", [P, 2, B_FULL], dt.float8e4, isOutput=False)
    ni_in = nc.declare_dram_parameter("negi", [P, P], dt.float8e4, isOutput=False)
    pi_in = nc.declare_dram_parameter("posi", [P, P], dt.float8e4, isOutput=False)
    srow_d = nc.declare_dram_parameter("srow", [P, NLSE * NQ], dt.float32, isOutput=True)
    scol_d = nc.declare_dram_parameter(
        "scol", [P, NLSE, 2, BLK], dt.bfloat16, isOutput=True
    )
    rmax_d = nc.declare_dram_parameter("rmax", [P, NQ * RMAX_SLOTS], dt.float32, isOutput=True)

    def emit_mms(ps, xT, qi, blk, c0, diag=None):
        """Gram matmuls for span (qi, blk) covering span cols [c0, 2048).

        diag=(negi, posi): fold a -57600*I correction into the 512-tile
        containing span cols [qi*128, qi*128+128) to kill self-matches.
        """
        lhs = xT[:, :, qi * P : (qi + 1) * P]
        c = c0
        while c < BLK:
            w = min(512 - (c % 512), BLK - c)
            col = blk * BLK + c
            has_diag = diag is not None and c <= qi * P < c + w
            nc.tensor.matmul(
                ps[:, c : c + w],
                lhs,
                xT[:, :, col : col + w],
                start=True,
                stop=not has_diag,
                perf_mode=DR,
            )
            if has_diag:
                nc.tensor.matmul(
                    ps[:, qi * P : (qi + 1) * P],
                    diag[0],
                    diag[1],
                    start=False,
                    stop=True,
                )
            c += w
        return ps

    # --- block-4 piece schedule -------------------------------------------
    # 64 pieces of (qi, pc) with pc in 0..3 covering cols [pc*512,(pc+1)*512)
    # of block 4.  8 "double" items merge (qi, pc even) + (qi, pc odd) into
    # one 1024-wide gram + single reduce.  Insertion points: LSE iterations
    # it = 8..63 (one item each).
    items = []  # (qi, c_start, width, slot_idx)
    for qi in (4, 5):  # 8 singles during blk0 iterations 8..15
        for pc in range(4):
            items.append((qi, pc * 512, 512, qi * 4 + pc))
    dq = [(0, 0), (0, 1024), (1, 0), (1, 1024), (2, 0), (2, 1024), (3, 0), (3, 1024)]
    singles = [(qi, pc * 512, 512, qi * 4 + pc) for qi in range(6, 16) for pc in range(4)]
    di = 0
    for k in range(48):  # iterations 16..63
        if k % 6 == 0 and di < 8:
            qi, c = dq[di]
            items.append((qi, c, 1024, qi * 4 + c // 512))
            di += 1
        else:
            items.append(singles.pop(0))
    assert not singles and di == 8 and len(items) == 56

    with tile.TileContext(nc) as tc:
        with (
            tc.tile_pool(name="persist", bufs=1) as persist,
            tc.tile_pool(name="ep", bufs=8) as ep,
            tc.tile_pool(name="sp", bufs=2) as sp,
            tc.tile_pool(name="ps", bufs=1, space="PSUM") as psp,
        ):
            # PSUM: two explicit 4-bank slots, hand ping-ponged
            psA = psp.tile([P, BLK], dt.float32, tag="psA")
            psB = psp.tile([P, BLK], dt.float32, tag="psB")
            slots = (psA, psB)

            ident = persist.tile([P, P], dt.float16)
            make_identity(nc, ident)
            wps = psA
            for _ in range(12):
                nc.tensor.matmul(wps[:, :P], ident, ident, start=True, stop=True)

            xT = persist.tile([P, 2, B_FULL], dt.float8e4)
            negi = persist.tile([P, P], dt.float8e4)
            nc.sync.dma_start(out=negi, in_=ni_in[:, :])
            posi = persist.tile([P, P], dt.float8e4)
            nc.sync.dma_start(out=posi, in_=pi_in[:, :])
            srow_sb = persist.tile([P, NLSE * NQ], dt.float32)
            rmax_sb = persist.tile([P, NQ * RMAX_SLOTS], dt.float32)
            nc.vector.memset(rmax_sb, -1.0e30)
            biasap = persist.tile([P, 1], dt.float32)
            nc.vector.memset(biasap, -BIAS)
            # preload the exp table set while DMA streams
            scratch1 = persist.tile([P, 1], dt.float32)
            nc.scalar.activation(
                out=scratch1, in_=biasap, func=AF.Exp, bias=biasap, scale=0.0
            )

            # stream input: blk0 upper half first (processed qi descending),
            # then blk0 lower, then block-4 cols (pieces start early), then
            # blocks 1..3, alternating two DMA queues.
            H = BLK // 2
            chunk_order = [H, 0, 8 * H, 9 * H, 2 * H, 3 * H, 4 * H, 5 * H, 6 * H, 7 * H]
            for i, ch in enumerate(chunk_order):
                q = nc.gpsimd if i % 2 == 0 else nc.sync
                q.dma_start(
                    out=xT[:, :, ch : ch + H],
                    in_=xq_in[:, :, ch : ch + H],
                )

            it = 0
            for blk in (0, 1, 2, 3):
                sblk = sp.tile([P, BLK], dt.bfloat16, tag="s")
                qis = range(NQ - 1, -1, -1) if blk == 0 else range(NQ)
                if blk == 0:
                    nc.vector.memset(sblk, 0.0)
                for qi in qis:
                    c0 = P * qi if blk == 0 else 0
                    s = slots[it % 2]
                    emit_mms(s, xT, qi, blk, c0, diag=(negi, posi) if blk == 0 else None)
                    e = ep.tile([P, BLK], dt.bfloat16, tag="e")
                    nc.scalar.activation(
                        out=e[:, c0:BLK],
                        in_=s[:, c0:BLK],
                        func=AF.Exp,
                        bias=biasap,
                        scale=ACT_SCALE,
                        accum_out=srow_sb[:, blk * NQ + qi : blk * NQ + qi + 1],
                    )
                    # block-4 piece rides in the opposite slot's tail region
                    if it >= 8 and items:
                        pqi, pc, w, ridx = items.pop(0)
                        o = slots[(it + 1) % 2]
                        r0 = BLK - w
                        lhs = xT[:, :, pqi * P : (pqi + 1) * P]
                        for sc in range(0, w, 512):
                            nc.tensor.matmul(
                                o[:, r0 + sc : r0 + sc + 512],
                                lhs,
                                xT[:, :, 4 * BLK + pc + sc : 4 * BLK + pc + sc + 512],
                                start=True,
                                stop=True,
                                perf_mode=DR,
                            )
                        nc.vector.tensor_reduce(
                            rmax_sb[:, ridx : ridx + 1],
                            o[:, r0:BLK],
                            axis=mybir.AxisListType.X,
                            op=OP.max,
                        )
                    if blk == 0:
                        nc.vector.tensor_tensor(
                            sblk[:, c0:BLK], e[:, c0:BLK], sblk[:, c0:BLK], OP.add
                        )
                    elif qi == 0:
                        nc.vector.tensor_copy(sblk, e)
                    else:
                        nc.vector.tensor_tensor(sblk, e, sblk, OP.add)
                    it += 1
                if blk < 3:
                    nc.sync.dma_start(out=scol_d[:, blk, 0, :], in_=sblk)
                else:
                    h = BLK // 2
                    nc.sync.dma_start(out=scol_d[:, blk, 0, :h], in_=sblk[:, :h])
                    nc.scalar.dma_start(out=scol_d[:, blk, 0, h:], in_=sblk[:, h:])

            nc.sync.dma_start(out=srow_d[:, :], in_=srow_sb)
            nc.scalar.dma_start(out=rmax_d[:, :], in_=rmax_sb)

    nc.compile()
    return nc


_NC_CACHE = {}


def _get_nc():
    if "nc" not in _NC_CACHE:
        _NC_CACHE["nc"] = build_nc()
    return _NC_CACHE["nc"]


LAST_RESULT = None


def kernel(student_output: np.ndarray) -> np.ndarray:
    import os

    import ml_dtypes
    from concourse.bass_utils import run_bass_kernel_spmd

    global LAST_RESULT
    x = np.ascontiguousarray(student_output, dtype=np.float32)
    assert x.shape == (B_FULL, D)

    norm = np.maximum(np.linalg.norm(x, axis=1, keepdims=True), 1e-12)
    xn = x / norm
    xq = (xn * SCALE_IN).astype(ml_dtypes.float8_e4m3)  # [B, 256]
    xT = np.ascontiguousarray(xq.T.reshape(2, P, B_FULL).transpose(1, 0, 2))

    ni = np.zeros((P, P), np.float32)
    np.fill_diagonal(ni, -240.0)
    pi = np.zeros((P, P), np.float32)
    np.fill_diagonal(pi, 240.0)
    ni = ni.astype(ml_dtypes.float8_e4m3)
    pi = pi.astype(ml_dtypes.float8_e4m3)

    nc = _get_nc()
    in_maps = [
        {"xq": np.roll(xT, -c * BLK, axis=2), "negi": ni, "posi": pi}
        for c in range(N_CORES)
    ]
    trace = bool(int(os.environ.get("KOLEO_TRACE", "0")))
    res = run_bass_kernel_spmd(
        nc, in_maps, core_ids=list(range(N_CORES)), trace=trace
    )
    LAST_RESULT = res

    S = np.zeros(B_FULL, np.float64)
    gmax4 = np.zeros(B_FULL, np.float64)
    for c in range(N_CORES):
        srow = np.asarray(res.results[c]["srow"], dtype=np.float64)  # [128, 64]
        scol = np.asarray(
            res.results[c]["scol"].astype(np.float32), dtype=np.float64
        )  # [128, 4, 2, 2048]
        rmax = np.asarray(res.results[c]["rmax"], dtype=np.float64)  # [128, 64]
        rp = srow.reshape(P, NLSE, NQ).sum(axis=1)  # [p, qi]
        rloc = rp.T.reshape(BLK)
        cloc = scol.sum(axis=(0, 2)).reshape(NLSE * BLK)
        mloc = rmax.reshape(P, NQ, RMAX_SLOTS).max(axis=2).T.reshape(BLK)
        base = c * BLK
        S[base : base + BLK] += rloc
        gmax4[base : base + BLK] = mloc / (SCALE_IN * SCALE_IN)
        for blk in range(NLSE):
            j0 = (base + blk * BLK) % B_FULL
            S[j0 : j0 + BLK] += cloc[blk * BLK : (blk + 1) * BLK]

    est = (np.log(S) + BIAS) / BETA
    g = np.maximum(est, gmax4)
    md = np.sqrt(np.clip(2.0 - 2.0 * g, 0.0, None))
    loss = -np.mean(np.log(md + 1e-8))
    return np.float32(loss)


if __name__ == "__main__":
    rng = np.random.default_rng(0)
    x = rng.standard_normal((B_FULL, D), dtype=np.float32)
    out = kernel(x)
    print("loss:", out)


# revision 11
# speedup vs baseline: 1.0253x; 1.0253x over previous
"""KoLeo-loss kernel v7 for 8 Trainium2 NeuronCores.

v6 -> v7:
  - walrus --enable-ldw-opt=true (monkeypatched into bass_utils
    run_command): consecutive matmuls sharing a stationary matrix get a
    single LDWEIGHTS, roughly halving PE time per LSE tile.
  - Block-4 pieces are scheduled so their row-tile (lhs) EQUALS the
    surrounding LSE iteration's qi, so gram spans + piece spans share
    one weight load.  qi 0..7 ride blocks 0..3 as four 512-col singles;
    qi 8..15 ride blocks 1/3 as singles and block 2 as a 1024 double.
  - Column-sum adds are delayed by one iteration so each piece's DVE
    reduce runs BEFORE the pending add, releasing the PSUM tail region
    before the next gram's last span needs it.
  - Explicit psA/psB PSUM slots (strict ping-pong, no pool rotation).

Scheme (unchanged): circulant cover, per-core 2048 rows.  Block 0
upper-triangular LSE with -240 diag kill; blocks 1..3 full LSE (exp
row-accum to srow, DVE column accumulation to scol); block 4 covered by
both endpoint cores, row-max only (rmax, 4 slots per qi, host
max-combines).  Host: S = srow + scol contributions, est = (log S +
BIAS)/BETA, gram-max = max(est, rmax/256), koleo loss.
"""

import sys

if "/opt/trn_rl_repo" not in sys.path:
    sys.path.insert(0, "/opt/trn_rl_repo")

import numpy as np

P = 128
D = 256
B_FULL = 16384
N_CORES = 8
BLK = 2048
NLSE = 4  # blocks 0..3 via LSE
NQ = 16

BETA = 768.0
BIAS = 227.0
SCALE_IN = 16.0
ACT_SCALE = BETA / (SCALE_IN * SCALE_IN)
RMAX_SLOTS = 4  # per-qi block-4 partial-max slots (512 cols each)


def _patch_ldw_opt():
    # walrus --enable-ldw-opt=true crashes in visitInstLdweights on
    # DoubleRow weight loads; keep the default (disabled).
    return


def build_nc():
    import concourse.mybir as mybir
    import concourse.tile as tile
    from concourse import bacc
    from concourse.masks import make_identity

    _patch_ldw_opt()

    dt = mybir.dt
    AF = mybir.ActivationFunctionType
    OP = mybir.AluOpType
    DR = mybir.MatmulPerfMode.DoubleRow

    nc = bacc.Bacc(None)
    xq_in = nc.declare_dram_parameter("xq", [P, 2, B_FULL], dt.float8e4, isOutput=False)
    ni_in = nc.declare_dram_parameter("negi", [P, P], dt.float8e4, isOutput=False)
    pi_in = nc.declare_dram_parameter("posi", [P, P], dt.float8e4, isOutput=False)
    srow_d = nc.declare_dram_parameter("srow", [P, NLSE * NQ], dt.float32, isOutput=True)
    scol_d = nc.declare_dram_parameter(
        "scol", [P, NLSE, 2, BLK], dt.bfloat16, isOutput=True
    )
    rmax_d = nc.declare_dram_parameter("rmax", [P, NQ * RMAX_SLOTS], dt.float32, isOutput=True)

    def emit_mms(ps, xT, qi, blk, c0, diag=None):
        """Gram matmuls for span (qi, blk) covering span cols [c0, 2048).

        diag=(negi, posi): fold a -57600*I correction into the 512-tile
        containing span cols [qi*128, qi*128+128) to kill self-matches.
        """
        lhs = xT[:, :, qi * P : (qi + 1) * P]
        c = c0
        while c < BLK:
            w = min(512 - (c % 512), BLK - c)
            col = blk * BLK + c
            has_diag = diag is not None and c <= qi * P < c + w
            nc.tensor.matmul(
                ps[:, c : c + w],
                lhs,
                xT[:, :, col : col + w],
                start=True,
                stop=not has_diag,
                perf_mode=DR,
            )
            if has_diag:
                nc.tensor.matmul(
                    ps[:, qi * P : (qi + 1) * P],
                    diag[0],
                    diag[1],
                    start=False,
                    stop=True,
                )
            c += w

    def piece_for(blk, qi):
        """Block-4 piece (c_start, width, rmax_slot) riding LSE tile (blk,qi)."""
        if qi <= 7:
            if blk == 0 and qi > 7:
                return None
            pc = blk  # blocks 0..3 carry pc 0..3
            return (pc * 512, 512, qi * RMAX_SLOTS + pc)
        if blk == 1:
            return (0, 512, qi * RMAX_SLOTS + 0)
        if blk == 2:
            return (1024, 1024, qi * RMAX_SLOTS + 2)
        if blk == 3:
            return (512, 512, qi * RMAX_SLOTS + 1)
        return None  # blk0 qi>7: no ride

    with tile.TileContext(nc) as tc:
        with (
            tc.tile_pool(name="persist", bufs=1) as persist,
            tc.tile_pool(name="ep", bufs=8) as ep,
            tc.tile_pool(name="sp", bufs=2) as sp,
            tc.tile_pool(name="ps", bufs=1, space="PSUM") as psp,
        ):
            psA = psp.tile([P, BLK], dt.float32, tag="psA")
            psB = psp.tile([P, BLK], dt.float32, tag="psB")
            slots = (psA, psB)

            ident = persist.tile([P, P], dt.float16)
            make_identity(nc, ident)
            for _ in range(12):
                nc.tensor.matmul(psA[:, :P], ident, ident, start=True, stop=True)

            xT = persist.tile([P, 2, B_FULL], dt.float8e4)
            negi = persist.tile([P, P], dt.float8e4)
            nc.sync.dma_start(out=negi, in_=ni_in[:, :])
            posi = persist.tile([P, P], dt.float8e4)
            nc.sync.dma_start(out=posi, in_=pi_in[:, :])
            srow_sb = persist.tile([P, NLSE * NQ], dt.float32)
            rmax_sb = persist.tile([P, NQ * RMAX_SLOTS], dt.float32)
            nc.vector.memset(rmax_sb, -1.0e30)
            biasap = persist.tile([P, 1], dt.float32)
            nc.vector.memset(biasap, -BIAS)
            # preload the exp table set while DMA streams
            scratch1 = persist.tile([P, 1], dt.float32)
            nc.scalar.activation(
                out=scratch1, in_=biasap, func=AF.Exp, bias=biasap, scale=0.0
            )

            # stream input: blk0 upper half first (processed qi descending),
            # then blk0 lower, then block-4 cols (pieces start at it=8), then
            # blocks 1..3, alternating the two DMA queues.
            H = BLK // 2
            chunk_order = [H, 0, 8 * H, 9 * H, 2 * H, 3 * H, 4 * H, 5 * H, 6 * H, 7 * H]
            for i, ch in enumerate(chunk_order):
                q = nc.gpsimd if i % 2 == 0 else nc.sync
                q.dma_start(
                    out=xT[:, :, ch : ch + H],
                    in_=xq_in[:, :, ch : ch + H],
                )

            it = 0
            for blk in (0, 1, 2, 3):
                sblk = sp.tile([P, BLK], dt.bfloat16, tag="s")
                qis = range(NQ - 1, -1, -1) if blk == 0 else range(NQ)
                if blk == 0:
                    nc.vector.memset(sblk, 0.0)
                pending = None
                for qi in qis:
                    c0 = P * qi if blk == 0 else 0
                    s = slots[it % 2]
                    emit_mms(s, xT, qi, blk, c0, diag=(negi, posi) if blk == 0 else None)
                    e = ep.tile([P, BLK], dt.bfloat16, tag="e")
                    nc.scalar.activation(
                        out=e[:, c0:BLK],
                        in_=s[:, c0:BLK],
                        func=AF.Exp,
                        bias=biasap,
                        scale=ACT_SCALE,
                        accum_out=srow_sb[:, blk * NQ + qi : blk * NQ + qi + 1],
                    )
                    pc = piece_for(blk, qi) if it >= 8 else None
                    if pc is not None:
                        cst, w, ridx = pc
                        o = slots[(it + 1) % 2]
                        r0 = BLK - w
                        lhs = xT[:, :, qi * P : (qi + 1) * P]
                        for sc in range(0, w, 512):
                            nc.tensor.matmul(
                                o[:, r0 + sc : r0 + sc + 512],
                                lhs,
                                xT[:, :, 4 * BLK + cst + sc : 4 * BLK + cst + sc + 512],
                                start=True,
                                stop=True,
                                perf_mode=DR,
                            )
                        nc.vector.tensor_reduce(
                            rmax_sb[:, ridx : ridx + 1],
                            o[:, r0:BLK],
                            axis=mybir.AxisListType.X,
                            op=OP.max,
                        )
                    if pending is not None:
                        pending()
                    if blk == 0:
                        pending = (
                            lambda e=e, c0=c0: nc.vector.tensor_tensor(
                                sblk[:, c0:BLK], e[:, c0:BLK], sblk[:, c0:BLK], OP.add
                            )
                        )
                    elif qi == 0:
                        pending = lambda e=e: nc.vector.tensor_copy(sblk, e)
                    else:
                        pending = lambda e=e: nc.vector.tensor_tensor(
                            sblk, e, sblk, OP.add
                        )
                    it += 1
                pending()
                if blk < 3:
                    nc.sync.dma_start(out=scol_d[:, blk, 0, :], in_=sblk)
                else:
                    h = BLK // 2
                    nc.sync.dma_start(out=scol_d[:, blk, 0, :h], in_=sblk[:, :h])
                    nc.scalar.dma_start(out=scol_d[:, blk, 0, h:], in_=sblk[:, h:])

            nc.sync.dma_start(out=srow_d[:, :], in_=srow_sb)
            nc.scalar.dma_start(out=rmax_d[:, :], in_=rmax_sb)

    nc.compile()
    return nc


_NC_CACHE = {}


def _get_nc():
    if "nc" not in _NC_CACHE:
        _NC_CACHE["nc"] = build_nc()
    return _NC_CACHE["nc"]


LAST_RESULT = None


def kernel(student_output: np.ndarray) -> np.ndarray:
    import os

    import ml_dtypes
    from concourse.bass_utils import run_bass_kernel_spmd

    global LAST_RESULT
    x = np.ascontiguousarray(student_output, dtype=np.float32)
    assert x.shape == (B_FULL, D)

    norm = np.maximum(np.linalg.norm(x, axis=1, keepdims=True), 1e-12)
    xn = x / norm
    xq = (xn * SCALE_IN).astype(ml_dtypes.float8_e4m3)  # [B, 256]
    xT = np.ascontiguousarray(xq.T.reshape(2, P, B_FULL).transpose(1, 0, 2))

    ni = np.zeros((P, P), np.float32)
    np.fill_diagonal(ni, -240.0)
    pi = np.zeros((P, P), np.float32)
    np.fill_diagonal(pi, 240.0)
    ni = ni.astype(ml_dtypes.float8_e4m3)
    pi = pi.astype(ml_dtypes.float8_e4m3)

    nc = _get_nc()
    in_maps = [
        {"xq": np.roll(xT, -c * BLK, axis=2), "negi": ni, "posi": pi}
        for c in range(N_CORES)
    ]
    trace = bool(int(os.environ.get("KOLEO_TRACE", "0")))
    res = run_bass_kernel_spmd(
        nc, in_maps, core_ids=list(range(N_CORES)), trace=trace
    )
    LAST_RESULT = res

    S = np.zeros(B_FULL, np.float64)
    gmax4 = np.zeros(B_FULL, np.float64)
    for c in range(N_CORES):
        srow = np.asarray(res.results[c]["srow"], dtype=np.float64)  # [128, 64]
        scol = np.asarray(
            res.results[c]["scol"].astype(np.float32), dtype=np.float64
        )  # [128, 4, 2, 2048]
        rmax = np.asarray(res.results[c]["rmax"], dtype=np.float64)  # [128, 64]
        rp = srow.reshape(P, NLSE, NQ).sum(axis=1)  # [p, qi]
        rloc = rp.T.reshape(BLK)
        cloc = scol.sum(axis=(0, 2)).reshape(NLSE * BLK)
        mloc = rmax.reshape(P, NQ, RMAX_SLOTS).max(axis=2).T.reshape(BLK)
        base = c * BLK
        S[base : base + BLK] += rloc
        gmax4[base : base + BLK] = mloc / (SCALE_IN * SCALE_IN)
        for blk in range(NLSE):
            j0 = (base + blk * BLK) % B_FULL
            S[j0 : j0 + BLK] += cloc[blk * BLK : (blk + 1) * BLK]

    est = (np.log(S) + BIAS) / BETA
    g = np.maximum(est, gmax4)
    md = np.sqrt(np.clip(2.0 - 2.0 * g, 0.0, None))
    loss = -np.mean(np.log(md + 1e-8))
    return np.float32(loss)


if __name__ == "__main__":
    rng = np.random.default_rng(0)
    x = rng.standard_normal((B_FULL, D), dtype=np.float32)
    out = kernel(x)
    print("loss:", out)


# revision 12
# speedup vs baseline: 1.2432x; 1.2125x over previous
"""KoLeo-loss kernel v8 for 8 Trainium2 NeuronCores.

v3 -> v8 (v5b structure, the piece-interleave experiments of v6/v7
regressed PE pstate and were reverted):
  - blk4 row-max tiles interleave every 3rd LSE iteration starting
    after blk0, emitted between the activation and the column add so
    the DVE queue runs the PSUM reduce before the pending add.
  - Input chunk order: blk0 upper half first (qi descending needs it),
    first two chunks on the fast HWDGE queues (sync + scalar), then
    alternate sync/gpsimd.
  - PE warmup 12 matmuls; ep pool 8 buffers.
  - Final column-sum DMA split across sync + scalar queues.

Scheme (unchanged from v3): circulant cover, per-core 2048 rows, fp8
DoubleRow gram.  Block 0 upper-triangular LSE with -240 diag kill;
blocks 1..3 full LSE (exp row-accum to srow, DVE column accumulation
to scol); block 4 covered by both endpoint cores, row-max only.
Host: S = srow + scol contributions, est = (log S + BIAS)/BETA,
gram-max = max(est, rmax/256), koleo loss.
"""

import sys

if "/opt/trn_rl_repo" not in sys.path:
    sys.path.insert(0, "/opt/trn_rl_repo")

import numpy as np

P = 128
D = 256
B_FULL = 16384
N_CORES = 8
BLK = 2048
NLSE = 4  # blocks 0..3 via LSE
NQ = 16

BETA = 768.0
BIAS = 227.0
SCALE_IN = 16.0
ACT_SCALE = BETA / (SCALE_IN * SCALE_IN)


def build_nc():
    import concourse.mybir as mybir
    import concourse.tile as tile
    from concourse import bacc
    from concourse.masks import make_identity

    dt = mybir.dt
    AF = mybir.ActivationFunctionType
    OP = mybir.AluOpType
    DR = mybir.MatmulPerfMode.DoubleRow

    nc = bacc.Bacc(None)
    xq_in = nc.declare_dram_parameter("xq", [P, 2, B_FULL], dt.float8e4, isOutput=False)
    ni_in = nc.declare_dram_parameter("negi", [P, P], dt.float8e4, isOutput=False)
    pi_in = nc.declare_dram_parameter("posi", [P, P], dt.float8e4, isOutput=False)
    srow_d = nc.declare_dram_parameter("srow", [P, NLSE * NQ], dt.float32, isOutput=True)
    scol_d = nc.declare_dram_parameter(
        "scol", [P, NLSE, 2, BLK], dt.bfloat16, isOutput=True
    )
    rmax_d = nc.declare_dram_parameter("rmax", [P, NQ], dt.float32, isOutput=True)

    def emit_mms(ps, xT, qi, blk, c0, diag=None):
        """Gram matmuls for span (qi, blk) covering span cols [c0, 2048).

        diag=(negi, posi): fold a -57600*I correction into the 512-tile
        containing span cols [qi*128, qi*128+128) to kill self-matches.
        """
        lhs = xT[:, :, qi * P : (qi + 1) * P]
        c = c0
        while c < BLK:
            w = min(512 - (c % 512), BLK - c)
            col = blk * BLK + c
            has_diag = diag is not None and c <= qi * P < c + w
            nc.tensor.matmul(
                ps[:, c : c + w],
                lhs,
                xT[:, :, col : col + w],
                start=True,
                stop=not has_diag,
                perf_mode=DR,
            )
            if has_diag:
                nc.tensor.matmul(
                    ps[:, qi * P : (qi + 1) * P],
                    diag[0],
                    diag[1],
                    start=False,
                    stop=True,
                )
            c += w

    with tile.TileContext(nc) as tc:
        with (
            tc.tile_pool(name="persist", bufs=1) as persist,
            tc.tile_pool(name="ep", bufs=8) as ep,
            tc.tile_pool(name="sp", bufs=2) as sp,
            tc.tile_pool(name="ps", bufs=2, space="PSUM") as psp,
        ):
            xT = persist.tile([P, 2, B_FULL], dt.float8e4)
            negi = persist.tile([P, P], dt.float8e4)
            posi = persist.tile([P, P], dt.float8e4)
            srow_sb = persist.tile([P, NLSE * NQ], dt.float32)
            rmax_sb = persist.tile([P, NQ], dt.float32)
            biasap = persist.tile([P, 1], dt.float32)
            nc.vector.memset(biasap, -BIAS)
            ident = persist.tile([P, P], dt.float16)
            make_identity(nc, ident)
            # preload the exp table set while DMA streams
            scratch1 = persist.tile([P, 1], dt.float32)
            nc.scalar.activation(
                out=scratch1, in_=biasap, func=AF.Exp, bias=biasap, scale=0.0
            )

            # stream input: blk0 upper half first (processed qi descending)
            # on the fast HWDGE queues, then the rest alternating.
            H = BLK // 2
            nc.sync.dma_start(out=xT[:, :, H : 2 * H], in_=xq_in[:, :, H : 2 * H])
            nc.scalar.dma_start(out=xT[:, :, 0:H], in_=xq_in[:, :, 0:H])
            nc.sync.dma_start(out=negi, in_=ni_in[:, :])
            nc.sync.dma_start(out=posi, in_=pi_in[:, :])
            for i, ch in enumerate(range(2 * H, 10 * H, H)):
                q = nc.gpsimd if i % 2 == 0 else nc.sync
                q.dma_start(
                    out=xT[:, :, ch : ch + H],
                    in_=xq_in[:, :, ch : ch + H],
                )

            wps = psp.tile([P, BLK], dt.float32, tag="ps", name="warm")
            for _ in range(12):
                nc.tensor.matmul(wps[:, :P], ident, ident, start=True, stop=True)

            def emit_blk4(qi):
                ps = psp.tile([P, BLK], dt.float32, tag="ps")
                emit_mms(ps, xT, qi, 4, 0)
                nc.vector.tensor_reduce(
                    rmax_sb[:, qi : qi + 1], ps, axis=mybir.AxisListType.X, op=OP.max
                )

            it = 0
            for blk in (0, 1, 2, 3):
                sblk = sp.tile([P, BLK], dt.bfloat16, tag="s")
                qis = range(NQ - 1, -1, -1) if blk == 0 else range(NQ)
                if blk == 0:
                    nc.vector.memset(sblk, 0.0)
                for qi in qis:
                    c0 = P * qi if blk == 0 else 0
                    ps = psp.tile([P, BLK], dt.float32, tag="ps")
                    emit_mms(ps, xT, qi, blk, c0, diag=(negi, posi) if blk == 0 else None)
                    e = ep.tile([P, BLK], dt.bfloat16, tag="e")
                    nc.scalar.activation(
                        out=e[:, c0:BLK],
                        in_=ps[:, c0:BLK],
                        func=AF.Exp,
                        bias=biasap,
                        scale=ACT_SCALE,
                        accum_out=srow_sb[:, blk * NQ + qi : blk * NQ + qi + 1],
                    )
                    if it >= 16 and (it - 16) % 3 == 0 and (it - 16) // 3 < NQ:
                        emit_blk4((it - 16) // 3)
                    if blk == 0:
                        nc.vector.tensor_tensor(
                            sblk[:, c0:BLK], e[:, c0:BLK], sblk[:, c0:BLK], OP.add
                        )
                    elif qi == 0:
                        nc.vector.tensor_copy(sblk, e)
                    else:
                        nc.vector.tensor_tensor(sblk, e, sblk, OP.add)
                    it += 1
                if blk < 3:
                    nc.sync.dma_start(out=scol_d[:, blk, 0, :], in_=sblk)
                else:
                    h = BLK // 2
                    nc.sync.dma_start(out=scol_d[:, blk, 0, :h], in_=sblk[:, :h])
                    nc.scalar.dma_start(out=scol_d[:, blk, 0, h:], in_=sblk[:, h:])

            nc.sync.dma_start(out=srow_d[:, :], in_=srow_sb)
            nc.scalar.dma_start(out=rmax_d[:, :], in_=rmax_sb)

    nc.compile()
    return nc


_NC_CACHE = {}


def _get_nc():
    if "nc" not in _NC_CACHE:
        _NC_CACHE["nc"] = build_nc()
    return _NC_CACHE["nc"]


LAST_RESULT = None


def kernel(student_output: np.ndarray) -> np.ndarray:
    import os

    import ml_dtypes
    from concourse.bass_utils import run_bass_kernel_spmd

    global LAST_RESULT
    x = np.ascontiguousarray(student_output, dtype=np.float32)
    assert x.shape == (B_FULL, D)

    norm = np.maximum(np.linalg.norm(x, axis=1, keepdims=True), 1e-12)
    xn = x / norm
    xq = (xn * SCALE_IN).astype(ml_dtypes.float8_e4m3)  # [B, 256]
    xT = np.ascontiguousarray(xq.T.reshape(2, P, B_FULL).transpose(1, 0, 2))

    ni = np.zeros((P, P), np.float32)
    np.fill_diagonal(ni, -240.0)
    pi = np.zeros((P, P), np.float32)
    np.fill_diagonal(pi, 240.0)
    ni = ni.astype(ml_dtypes.float8_e4m3)
    pi = pi.astype(ml_dtypes.float8_e4m3)

    nc = _get_nc()
    in_maps = [
        {"xq": np.roll(xT, -c * BLK, axis=2), "negi": ni, "posi": pi}
        for c in range(N_CORES)
    ]
    trace = bool(int(os.environ.get("KOLEO_TRACE", "0")))
    res = run_bass_kernel_spmd(
        nc, in_maps, core_ids=list(range(N_CORES)), trace=trace
    )
    LAST_RESULT = res

    S = np.zeros(B_FULL, np.float64)
    gmax4 = np.zeros(B_FULL, np.float64)
    for c in range(N_CORES):
        srow = np.asarray(res.results[c]["srow"], dtype=np.float64)  # [128, 64]
        scol = np.asarray(
            res.results[c]["scol"].astype(np.float32), dtype=np.float64
        )  # [128, 4, 2, 2048]
        rmax = np.asarray(res.results[c]["rmax"], dtype=np.float64)  # [128, 16]
        rp = srow.reshape(P, NLSE, NQ).sum(axis=1)  # [p, qi]
        rloc = rp.T.reshape(BLK)
        cloc = scol.sum(axis=(0, 2)).reshape(NLSE * BLK)
        mloc = rmax.T.reshape(BLK)  # local row qi*128+p
        base = c * BLK
        S[base : base + BLK] += rloc
        gmax4[base : base + BLK] = mloc / (SCALE_IN * SCALE_IN)
        for blk in range(NLSE):
            j0 = (base + blk * BLK) % B_FULL
            S[j0 : j0 + BLK] += cloc[blk * BLK : (blk + 1) * BLK]

    est = (np.log(S) + BIAS) / BETA
    g = np.maximum(est, gmax4)
    md = np.sqrt(np.clip(2.0 - 2.0 * g, 0.0, None))
    loss = -np.mean(np.log(md + 1e-8))
    return np.float32(loss)


if __name__ == "__main__":
    rng = np.random.default_rng(0)
    x = rng.standard_normal((B_FULL, D), dtype=np.float32)
    out = kernel(x)
    print("loss:", out)
